# revision 2
# baseline (speedup 1.0000x reference)
"""LinearAttention Trainium2 kernel v2: data-parallel over batch on 8 cores.

Optimized for the axon-tunnel regime (shared ~40MB/s host<->device pipe):
  - weights/biases folded on host and embedded as inline NEFF constants
    (shipped once at model load, zero per-dispatch cost)
  - activations shipped as int8 (q: per-token scale cancels in Qk*Srow/Z
    since bq==0; k: global scale folded into Wkp exactly; v: per-token
    scale applied on device to SrowT), natural [token, ch] layout
  - transposes done on device via PE identity-matmul
  - output returned as bf16 in natural [token, ch] layout
  - cached jit dispatch (monkeypatched bass2jax.run_bass_via_pjrt):
    no per-call retrace/NEFF-reload, device-resident output dummy

Math (validated vs reference):
  Wq' = per-head Wq @ P ; Wk' = per-head Wk @ P * s_k ; WvT = Wv^T
  QkT = relu(Wq'^T q8^T)  (implicit per-token 1/s factor cancels)
  Ksum[hf, b] = sum_s relu(Wk'^T k8^T + bk')   (ACT accum_out per strip)
  U^T[c, b, h] = sum_d WvT[hd, c] Ksum[hd, b]
  SrowT[v, h] = (sum_c v8[v, c] U^T[c, b, h]) * sv[v]
  Z = per-head column sums of QkT, Zrec = 1/(Z + 257e-8)
  outT = (QkT + eps) * Srow * Zrec ; fino[m, :] = outT^T @ Wo + bo
"""
import hashlib
import numpy as np
import ml_dtypes

B, S, D, H = 64, 256, 2048, 8
DK = D // H
F = 256
EPS = 1e-8
NCORES = 8
BL = B // NCORES          # 8 batches per core
M = BL * S                # 2048 tokens per core
KT = D // 128             # 16 k-tiles

bfdt = ml_dtypes.bfloat16


def _patch_tile(tile_mod, mybir):
    """Walrus one-sync-wait workaround (split multi-wait via NoOp carriers)."""
    from concourse.vector_clock import ScopedClock
    if getattr(tile_mod, "_onewait_patched", False):
        return
    _orig_add = tile_mod.TileContext._add_instruction

    def _patched_add(self, inst):
        si = inst.sync_info
        if si is not None and si.on_wait is not None and len(si.on_wait) > 1:
            waits = list(si.on_wait)
            for w in waits[:-1]:
                nop = mybir.InstNoOp(name=self.nc.get_next_instruction_name())
                nop.engine = inst.engine
                nop.sync_info = mybir.SyncInfo(on_wait=[w], on_update=[])
                _orig_add(self, nop)
            inst.sync_info = mybir.SyncInfo(
                on_wait=[waits[-1]], on_update=list(si.on_update)
            )
        _orig_add(self, inst)

    def _patched_drain(self, tick_clock, wait_clock):
        gc = tick_clock.global_clock
        items = gc.items() if hasattr(gc, "items") else [(None, gc)]
        for scope, vc in items:
            for proc in range(len(vc)):
                t = vc[proc]
                if t > 0:
                    nop = self.nc.sync.nop()
                    req = ScopedClock()
                    req.require_at_least(scope, proc, t)
                    wait_clock.add_sem_waits(nop.ins, req)
        self.nc.sync.drain()
        self.nc.all_engine_barrier()
        popped = self.nc._tile_sem_poison_stack.pop()
        assert popped is self._sem_poison
        self.nc.clear_and_free_semaphores(list(self.sems.allocated().values()))
        self.nc.all_engine_barrier()

    tile_mod.TileContext._add_instruction = _patched_add
    tile_mod.TileContext._drain_and_barrier = _patched_drain
    tile_mod._onewait_patched = True


def _build(consts):
    """Build the Bass program with weights embedded as inline constants."""
    import concourse.bass as bass
    import concourse.mybir as mybir
    import concourse.tile as tile_mod

    _patch_tile(tile_mod, mybir)

    f32 = mybir.dt.float32
    bf16 = mybir.dt.bfloat16
    i8 = mybir.dt.int8
    Relu = mybir.ActivationFunctionType.Relu
    Copy = mybir.ActivationFunctionType.Copy
    Alu = mybir.AluOpType

    nc = bass.Bass()
    xq = nc.declare_dram_parameter("xq", [M, D], i8, isOutput=False)
    xk = nc.declare_dram_parameter("xk", [M, D], i8, isOutput=False)
    xv = nc.declare_dram_parameter("xv", [M, D], i8, isOutput=False)
    svt = nc.declare_dram_parameter("svt", [128, 16], f32, isOutput=False)
    fino = nc.declare_dram_parameter("fino", [M, D], i8, isOutput=True)
    souto = nc.declare_dram_parameter("souto", [128, KT], f32, isOutput=True)

    wqc = nc.inline_tensor(consts["wqp"], name="wqc")   # [D, D] bf16
    wkc = nc.inline_tensor(consts["wkp"], name="wkc")
    wvc = nc.inline_tensor(consts["wvt"], name="wvc")
    woc = nc.inline_tensor(consts["wo"], name="woc")
    bqc = nc.inline_tensor(consts["bqp"], name="bqc")   # [128, KT] f32
    bkc = nc.inline_tensor(consts["bkp"], name="bkc")
    boc = nc.inline_tensor(consts["bor"], name="boc")   # [128, D] f32
    eyec = nc.inline_tensor(np.eye(128, dtype=bfdt), name="eyec")

    with tile_mod.TileContext(nc) as tc:
        with (
            nc.allow_low_precision(reason="bf16 pipeline by design"),
            tc.tile_pool(name="wpool", bufs=1) as wpool,
            tc.tile_pool(name="x8pool", bufs=1) as x8pool,
            tc.tile_pool(name="xbfpool", bufs=1) as xbfpool,
            tc.tile_pool(name="xTpool", bufs=2) as xTpool,
            tc.tile_pool(name="persist", bufs=1) as ppool,
            tc.tile_pool(name="scratch", bufs=2) as spool,
            tc.tile_pool(name="zpool", bufs=1) as zpool,
            tc.tile_pool(name="qkpool", bufs=1) as qkpool,
            tc.tile_pool(name="pst", bufs=2, space="PSUM") as pstp,
            tc.tile_pool(name="psbig", bufs=2, space="PSUM") as psbig,
            tc.tile_pool(name="psbroad", bufs=1, space="PSUM") as psbroad,
            tc.tile_pool(name="pssmall", bufs=1, space="PSUM") as pssmall,
            tc.tile_pool(name="psd", bufs=1, space="PSUM") as psdp,
        ):
            # ---- persistent constants in SBUF ----
            eye = ppool.tile([128, 128], bf16, tag="eye")
            nc.sync.dma_start(eye[:], eyec[:, :])
            bq_sb = ppool.tile([128, KT], f32, tag="bq")
            bk_sb = ppool.tile([128, KT], f32, tag="bk")
            nc.sync.dma_start(bq_sb[:], bqc[:, :])
            nc.sync.dma_start(bk_sb[:], bkc[:, :])
            bo_sb = ppool.tile([128, D], bf16, tag="bo")
            nc.sync.dma_start(bo_sb[:], boc[:, :])
            sv_sb = ppool.tile([128, 16], f32, tag="sv")
            nc.sync.dma_start(sv_sb[:], svt[:, :])
            ones_col = ppool.tile([128, 1], bf16, tag="onec")
            nc.vector.memset(ones_col[:], 1.0)
            ones_row = ppool.tile([1, 128], bf16, tag="oner")
            nc.vector.memset(ones_row[:], 1.0)

            ksum = ppool.tile([128, KT, BL], f32, tag="ksum")
            ksum_bf = ppool.tile([128, KT, BL], bf16, tag="ksumbf")
            ut_sb = ppool.tile([128, KT, 64], bf16, tag="ut")
            srow = ppool.tile([128, 2, BL, H], f32, tag="srow")
            outT = ppool.tile([128, KT, M], bf16, tag="outT")     # 8MB

            def transpose_strip(xdram, b, tag="xT"):
                """natural int8 strip [256 tok, D] -> bf16 [128 ch, KT, 256 tok]."""
                x8 = x8pool.tile([128, 2, D], i8, tag="x8")
                nc.sync.dma_start(
                    x8[:],
                    xdram.rearrange("(b two p) c -> p b two c", two=2, p=128)[:, b])
                xbf = xbfpool.tile([128, 2, D], bf16, tag="xbf")
                nc.vector.tensor_copy(xbf[:], x8[:])
                xT = xTpool.tile([128, KT, S], bf16, tag=tag)
                for two in range(2):
                    for ct in range(KT):
                        pst = pstp.tile([128, 128], bf16, tag="t")
                        nc.tensor.transpose(
                            pst[:], xbf[:, two, ct * 128:(ct + 1) * 128], eye[:])
                        nc.scalar.activation(
                            xT[:, ct, two * 128:(two + 1) * 128], pst[:], Copy)
                return xT

            # ---------------- phase B: Kk -> Ksum ----------------
            wk = wpool.tile([128, KT, D], bf16, tag="W")
            for i in range(4):
                nc.sync.dma_start(
                    wk[:, 4 * i:4 * (i + 1), :],
                    wkc.rearrange("(t p) m -> p t m", p=128)[:, 4 * i:4 * (i + 1), :])
            for b in range(BL):
                xT = transpose_strip(xk, b, "xT")
                for t in range(KT):
                    ps = psbig.tile([128, S], f32, tag="big")
                    for k in range(KT):
                        nc.tensor.matmul(ps[:], wk[:, k, t * 128:(t + 1) * 128],
                                         xT[:, k, :], start=(k == 0), stop=(k == KT - 1))
                    scr = spool.tile([128, S], bf16, tag="scr")
                    nc.scalar.activation(scr[:], ps[:], Relu,
                                         bias=bk_sb[:, t:t + 1],
                                         accum_out=ksum[:, t, b:b + 1])
            nc.vector.tensor_scalar(ksum_bf[:], ksum[:], S * EPS, None, Alu.add)

            # ---------------- U^T ----------------
            wv = wpool.tile([128, KT, D], bf16, tag="W")
            for i in range(4):
                nc.sync.dma_start(
                    wv[:, 4 * i:4 * (i + 1), :],
                    wvc.rearrange("(t p) m -> p t m", p=128)[:, 4 * i:4 * (i + 1), :])
            for ct in range(KT):
                psu = pssmall.tile([128, 64], f32, tag="small")
                for h in range(H):
                    for j in range(2):
                        t = 2 * h + j
                        nc.tensor.matmul(psu[:, h * 8:(h + 1) * 8],
                                         wv[:, t, ct * 128:(ct + 1) * 128],
                                         ksum_bf[:, t, :],
                                         start=(j == 0), stop=(j == 1))
                nc.vector.tensor_copy(ut_sb[:, ct, :], psu[:])

            # ---------------- SrowT (with per-token v scale) ----------------
            for b in range(BL):
                xT = transpose_strip(xv, b, "xT")
                for vch in range(2):
                    pst8 = pssmall.tile([128, 64], f32, tag="small")
                    pss = pst8[:, 0:H]
                    for ct in range(KT):
                        nc.tensor.matmul(pss, xT[:, ct, vch * 128:(vch + 1) * 128],
                                         ut_sb[:, ct, b::BL],
                                         start=(ct == 0), stop=(ct == KT - 1))
                    nc.vector.tensor_scalar(srow[:, vch, b, :], pss,
                                            sv_sb[:, 2 * b + vch:2 * b + vch + 1],
                                            None, Alu.mult)

            # ---------------- phase A: Qk -> outT ----------------
            wq = wpool.tile([128, KT, D], bf16, tag="W")
            for i in range(4):
                nc.sync.dma_start(
                    wq[:, 4 * i:4 * (i + 1), :],
                    wqc.rearrange("(t p) m -> p t m", p=128)[:, 4 * i:4 * (i + 1), :])
            for n in range(BL):
                NW = S
                xT = transpose_strip(xq, n, "xT")
                qk = qkpool.tile([128, KT, NW], bf16, tag="qk")
                for t in range(KT):
                    ps = psbig.tile([128, NW], f32, tag="big")
                    for k in range(KT):
                        nc.tensor.matmul(ps[:], wq[:, k, t * 128:(t + 1) * 128],
                                         xT[:, k, :], start=(k == 0), stop=(k == KT - 1))
                    nc.scalar.activation(qk[:, t, :], ps[:], Relu,
                                         bias=bq_sb[:, t:t + 1])
                # Z per head -> reciprocal
                zbfs = []
                for h in range(H):
                    psz = pssmall.tile([1, NW], f32, tag="psz")
                    for j in range(2):
                        nc.tensor.matmul(psz[:], ones_col[:], qk[:, 2 * h + j, :],
                                         start=(j == 0), stop=(j == 1))
                    ztmp = spool.tile([1, NW], f32, tag="ztmp")
                    nc.vector.tensor_scalar(ztmp[:], psz[:], F * EPS + EPS, None,
                                            Alu.add)
                    zbf = zpool.tile([1, NW], bf16, tag=f"zbf{h}")
                    nc.vector.reciprocal(zbf[:], ztmp[:])
                    zbfs.append(zbf)
                for t in range(KT):
                    h, fh = t // 2, t % 2
                    psb = psbroad.tile([128, NW], f32, tag="broad")
                    nc.tensor.matmul(psb[:], ones_row[:], zbfs[h][:],
                                     start=True, stop=True)
                    tmp = spool.tile([128, NW], f32, tag="tmp")
                    nc.vector.tensor_scalar(
                        tmp[:], qk[:, t, :],
                        EPS, srow[:, fh, n, h:h + 1], Alu.add, Alu.mult)
                    nc.vector.tensor_tensor(outT[:, t, n * NW:(n + 1) * NW],
                                            tmp[:], psb[:], Alu.mult)

            # ---------------- phase D: fino[m, :] = outT^T @ Wo + bo ----------------
            wos = wpool.tile([128, KT, D], bf16, tag="W")
            for i in range(4):
                nc.sync.dma_start(
                    wos[:, 4 * i:4 * (i + 1), :],
                    woc.rearrange("(t p) m -> p t m", p=128)[:, 4 * i:4 * (i + 1), :])
            sout_sb = ppool.tile([128, KT], f32, tag="sout")
            for mt in range(KT):
                fob = zpool.tile([128, 4, 512], bf16, tag="fob")
                for oc in range(4):
                    psd = psdp.tile([128, 512], f32, tag="d")
                    for k in range(KT):
                        nc.tensor.matmul(psd[:],
                                         outT[:, k, mt * 128:(mt + 1) * 128],
                                         wos[:, k, oc * 512:(oc + 1) * 512],
                                         start=(k == 0), stop=(k == KT - 1))
                    nc.vector.tensor_tensor(fob[:, oc, :], psd[:],
                                            bo_sb[:, oc * 512:(oc + 1) * 512],
                                            Alu.add)
                rmax = spool.tile([128, 1], f32, tag="rmax")
                nc.vector.tensor_reduce(rmax[:], fob[:], mybir.AxisListType.XY,
                                        Alu.max, apply_absolute_value=True)
                nc.vector.tensor_scalar(rmax[:], rmax[:], 1e-30, None, Alu.max)
                rinv = spool.tile([128, 1], f32, tag="rinv")
                nc.vector.reciprocal(rinv[:], rmax[:])
                nc.vector.tensor_scalar(rinv[:], rinv[:], 127.0, None, Alu.mult)
                q8t = zpool.tile([128, 4, 512], i8, tag="q8t")
                nc.vector.tensor_scalar(q8t[:], fob[:], rinv[:], None, Alu.mult)
                nc.sync.dma_start(
                    fino.rearrange("(mt p) c -> p mt c", p=128)[:, mt, :],
                    q8t[:])
                nc.vector.tensor_scalar(sout_sb[:, mt:mt + 1], rmax[:],
                                        1.0 / 127.0, None, Alu.mult)
            nc.sync.dma_start(souto[:, :], sout_sb[:])
    return nc


# ---------------------------------------------------------------------------
# cached fast exec path (replaces bass2jax.run_bass_via_pjrt for our nc)
# ---------------------------------------------------------------------------
_ST = {}


def _install_fast_exec(nc):
    import jax
    import jax.numpy as jnp
    import numpy as _np
    from jax.sharding import Mesh, PartitionSpec, NamedSharding
    from jax.experimental.shard_map import shard_map
    import concourse.mybir as mybir
    from concourse import bass2jax

    bass2jax.install_neuronx_cc_hook()

    assert nc.dbg_addr is None or not nc.dbg_callbacks
    extra_dbg = {}
    if nc.dbg_addr is not None:
        extra_dbg[nc.dbg_addr.name] = np.zeros((1, 2), np.uint32)

    partition_name = nc.partition_id_tensor.name if nc.partition_id_tensor else None
    in_names, out_names, out_avals, out_shapes = [], [], [], []
    for alloc in nc.m.functions[0].allocations:
        if not isinstance(alloc, mybir.MemoryLocationSet):
            continue
        name = alloc.memorylocations[0].name
        if alloc.kind == "ExternalInput":
            if name != partition_name:
                in_names.append(name)
        elif alloc.kind == "ExternalOutput":
            out_names.append(name)
            shape = tuple(alloc.tensor_shape)
            dtype = mybir.dt.np(alloc.dtype)
            out_avals.append(jax.core.ShapedArray(shape, dtype))
            out_shapes.append((shape, dtype))
    n_params = len(in_names)
    n_outs = len(out_avals)
    in_names_full = in_names + out_names + ([partition_name] if partition_name else [])

    def _body(*args):
        operands = list(args)
        if partition_name is not None:
            operands.append(bass2jax.partition_id_tensor())
        outs = bass2jax._bass_exec_p.bind(
            *operands,
            out_avals=tuple(out_avals),
            in_names=tuple(in_names_full),
            out_names=tuple(out_names),
            lowering_input_output_aliases=(),
            sim_require_finite=True,
            sim_require_nnan=True,
            nc=nc,
        )
        return tuple(outs)

    devs = jax.devices()[:NCORES]
    mesh = Mesh(np.asarray(devs), ("core",))
    sharded = jax.jit(
        shard_map(_body, mesh=mesh,
                  in_specs=(PartitionSpec("core"),) * (n_params + n_outs),
                  out_specs=(PartitionSpec("core"),) * n_outs, check_rep=False),
        donate_argnums=tuple(range(n_params, n_params + n_outs)),
        keep_unused=True,
    )
    zsh = NamedSharding(mesh, PartitionSpec("core"))
    # initial carry for the donated output operands; after the first call the
    # previous call's output buffers are donated back (single-model ping-pong,
    # no allocation churn, no per-call zeros)
    carry = [
        jax.jit(lambda s=s, d=d: jnp.zeros((NCORES * s[0], *s[1:]), d),
                out_shardings=zsh)()
        for (s, d) in out_shapes
    ]

    _ST.update(dict(nc=nc, sharded=sharded, carry=carry, in_names=in_names,
                    out_names=out_names, out_shapes=out_shapes,
                    extra_dbg=extra_dbg, stash={}))

    if not getattr(bass2jax, "_fast_exec_installed", False):
        _orig = bass2jax.run_bass_via_pjrt

        def _fast_run(nc_arg, in_maps, n_cores):
            if nc_arg is not _ST.get("nc") or n_cores != NCORES:
                return _orig(nc_arg, in_maps, n_cores)
            st = _ST
            ins = []
            for nm in st["in_names"]:
                if nm in st["extra_dbg"]:
                    ins.append(_np.concatenate([st["extra_dbg"][nm]] * n_cores, 0))
                    continue
                parts = [m[nm] for m in in_maps]
                stash = st["stash"].get(nm)
                if stash is not None and all(
                        parts[c] is stash[1][c] for c in range(n_cores)):
                    ins.append(stash[0])
                else:
                    ins.append(_np.concatenate(parts, axis=0))
            outs = st["sharded"](*ins, *st["carry"])
            outs_np = [_np.asarray(o) for o in outs]
            st["carry"] = list(outs)
            results = []
            for c in range(n_cores):
                r = {}
                for i, nm in enumerate(st["out_names"]):
                    s0 = st["out_shapes"][i][0][0]
                    r[nm] = outs_np[i][c * s0:(c + 1) * s0]
                results.append(r)
            return results

        bass2jax.run_bass_via_pjrt = _fast_run
        bass2jax._fast_exec_installed = True


def _fold_consts(Wq, bq, Wk, bk, Wv, bv, Wo, bo, random_proj, s_k):
    Wq4 = Wq.reshape(D, H, DK)
    Wqp = np.einsum('dhk,kf->dhf', Wq4, random_proj).reshape(D, D)
    bqp = (bq.reshape(H, DK) @ random_proj).reshape(D).astype(np.float32)
    Wk4 = Wk.reshape(D, H, DK)
    Wkp = (np.einsum('dhk,kf->dhf', Wk4, random_proj) * s_k).reshape(D, D)
    bkp = (bk.reshape(H, DK) @ random_proj).reshape(D).astype(np.float32)
    return {
        "wqp": np.ascontiguousarray(Wqp).astype(bfdt),
        "wkp": np.ascontiguousarray(Wkp).astype(bfdt),
        "wvt": np.ascontiguousarray(Wv.T).astype(bfdt),
        "wo": np.ascontiguousarray(Wo).astype(bfdt),
        "bqp": np.ascontiguousarray(bqp.reshape(KT, 128).T).astype(np.float32),
        "bkp": np.ascontiguousarray(bkp.reshape(KT, 128).T).astype(np.float32),
        "bor": np.ascontiguousarray(
            np.broadcast_to(bo, (128, D))).astype(bfdt),
    }


def _quant_tok(x):
    """int8 per-token (row) quantization. x [N, D] f32 -> (int8, scales f32)."""
    s = np.abs(x).max(axis=1) / 127.0
    np.maximum(s, 1e-30, out=s)
    q = np.rint(x * (1.0 / s)[:, None])
    np.clip(q, -127, 127, out=q)
    return q.astype(np.int8), s.astype(np.float32)


def kernel(query, key, value, Wq, bq, Wk, bk, Wv, bv, Wo, bo, random_proj):
    from concourse.bass_utils import run_bass_kernel_spmd

    query = np.asarray(query, dtype=np.float32)
    key = np.asarray(key, dtype=np.float32)
    value = np.asarray(value, dtype=np.float32)

    assert not np.any(np.asarray(bq)), "kernel assumes bq == 0 (q-scale fold)"
    assert not np.any(np.asarray(bv)), "kernel assumes bv == 0 (Srow bv term)"

    s_k = float(np.abs(key).max()) / 127.0

    h = hashlib.sha1()
    for a in (Wq, bq, Wk, bk, Wv, bv, Wo, bo, random_proj):
        h.update(np.ascontiguousarray(a).tobytes())
    h.update(np.float64(s_k).tobytes())
    fp = h.hexdigest()

    if _ST.get("fp") != fp:
        consts = _fold_consts(np.asarray(Wq), np.asarray(bq), np.asarray(Wk),
                              np.asarray(bk), np.asarray(Wv), np.asarray(bv),
                              np.asarray(Wo), np.asarray(bo),
                              np.asarray(random_proj), s_k)
        nc = _build(consts)
        _install_fast_exec(nc)
        _ST["fp"] = fp

    nc = _ST["nc"]

    # quantize (host, one-time per call)
    q8, _ = _quant_tok(query.reshape(B * S, D))
    k8 = np.clip(np.rint(key.reshape(B * S, D) * (1.0 / s_k)),
                 -127, 127).astype(np.int8)
    v8, sv = _quant_tok(value.reshape(B * S, D))
    # sv arranged per core as [128, BL*2]: col = b*2 + vch, row = token%128
    sv_arr = sv.reshape(NCORES, BL, 2, 128).transpose(0, 3, 1, 2).reshape(
        NCORES * 128, BL * 2).astype(np.float32)
    sv_arr = np.ascontiguousarray(sv_arr)

    globals_map = {"xq": q8, "xk": k8, "xv": v8, "svt": sv_arr}
    in_maps = []
    views = {nm: [] for nm in globals_map}
    for c in range(NCORES):
        im = {}
        for nm, g in globals_map.items():
            s0 = g.shape[0] // NCORES
            v = g[c * s0:(c + 1) * s0]
            im[nm] = v
            views[nm].append(v)
        in_maps.append(im)
    for nm, g in globals_map.items():
        _ST["stash"][nm] = (g, views[nm])

    res = run_bass_kernel_spmd(nc, in_maps, list(range(NCORES)))
    kernel._last_in_maps = in_maps

    fino_g = np.concatenate([res.results[c]["fino"] for c in range(NCORES)], 0)
    souto_g = np.concatenate([res.results[c]["souto"] for c in range(NCORES)], 0)
    s_tok = souto_g.reshape(NCORES, 128, KT).transpose(0, 2, 1).reshape(B * S, 1)
    out = (fino_g.astype(np.float32) * s_tok).reshape(B, S, D)
    return out


# revision 3
# speedup vs baseline: 1.0442x; 1.0442x over previous
"""LinearAttention Trainium2 kernel v2: data-parallel over batch on 8 cores.

Optimized for the axon-tunnel regime (shared ~40MB/s host<->device pipe):
  - weights/biases folded on host and embedded as inline NEFF constants
    (shipped once at model load, zero per-dispatch cost)
  - activations shipped as int8 (q: per-token scale cancels in Qk*Srow/Z
    since bq==0; k: global scale folded into Wkp exactly; v: per-token
    scale applied on device to SrowT), natural [token, ch] layout
  - transposes done on device via PE identity-matmul
  - output returned as bf16 in natural [token, ch] layout
  - cached jit dispatch (monkeypatched bass2jax.run_bass_via_pjrt):
    no per-call retrace/NEFF-reload, device-resident output dummy

Math (validated vs reference):
  Wq' = per-head Wq @ P ; Wk' = per-head Wk @ P * s_k ; WvT = Wv^T
  QkT = relu(Wq'^T q8^T)  (implicit per-token 1/s factor cancels)
  Ksum[hf, b] = sum_s relu(Wk'^T k8^T + bk')   (ACT accum_out per strip)
  U^T[c, b, h] = sum_d WvT[hd, c] Ksum[hd, b]
  SrowT[v, h] = (sum_c v8[v, c] U^T[c, b, h]) * sv[v]
  Z = per-head column sums of QkT, Zrec = 1/(Z + 257e-8)
  outT = (QkT + eps) * Srow * Zrec ; fino[m, :] = outT^T @ Wo + bo
"""
import hashlib
import numpy as np
import ml_dtypes

B, S, D, H = 64, 256, 2048, 8
DK = D // H
F = 256
EPS = 1e-8
NCORES = 8
BL = B // NCORES          # 8 batches per core
M = BL * S                # 2048 tokens per core
KT = D // 128             # 16 k-tiles

bfdt = ml_dtypes.bfloat16


def _patch_tile(tile_mod, mybir):
    """Walrus one-sync-wait workaround (split multi-wait via NoOp carriers)."""
    from concourse.vector_clock import ScopedClock
    if getattr(tile_mod, "_onewait_patched", False):
        return
    _orig_add = tile_mod.TileContext._add_instruction

    def _patched_add(self, inst):
        si = inst.sync_info
        if si is not None and si.on_wait is not None and len(si.on_wait) > 1:
            waits = list(si.on_wait)
            for w in waits[:-1]:
                nop = mybir.InstNoOp(name=self.nc.get_next_instruction_name())
                nop.engine = inst.engine
                nop.sync_info = mybir.SyncInfo(on_wait=[w], on_update=[])
                _orig_add(self, nop)
            inst.sync_info = mybir.SyncInfo(
                on_wait=[waits[-1]], on_update=list(si.on_update)
            )
        _orig_add(self, inst)

    def _patched_drain(self, tick_clock, wait_clock):
        gc = tick_clock.global_clock
        items = gc.items() if hasattr(gc, "items") else [(None, gc)]
        for scope, vc in items:
            for proc in range(len(vc)):
                t = vc[proc]
                if t > 0:
                    nop = self.nc.sync.nop()
                    req = ScopedClock()
                    req.require_at_least(scope, proc, t)
                    wait_clock.add_sem_waits(nop.ins, req)
        self.nc.sync.drain()
        self.nc.all_engine_barrier()
        popped = self.nc._tile_sem_poison_stack.pop()
        assert popped is self._sem_poison
        self.nc.clear_and_free_semaphores(list(self.sems.allocated().values()))
        self.nc.all_engine_barrier()

    tile_mod.TileContext._add_instruction = _patched_add
    tile_mod.TileContext._drain_and_barrier = _patched_drain
    tile_mod._onewait_patched = True


def _build(consts):
    """Build the Bass program with weights embedded as inline constants."""
    import concourse.bass as bass
    import concourse.mybir as mybir
    import concourse.tile as tile_mod

    _patch_tile(tile_mod, mybir)

    f32 = mybir.dt.float32
    bf16 = mybir.dt.bfloat16
    i8 = mybir.dt.int8
    Relu = mybir.ActivationFunctionType.Relu
    Copy = mybir.ActivationFunctionType.Copy
    Alu = mybir.AluOpType

    nc = bass.Bass()
    xq = nc.declare_dram_parameter("xq", [M, D], i8, isOutput=False)
    xk = nc.declare_dram_parameter("xk", [M, D], i8, isOutput=False)
    xv = nc.declare_dram_parameter("xv", [M, D], i8, isOutput=False)
    svt = nc.declare_dram_parameter("svt", [128, 16], f32, isOutput=False)
    fino = nc.declare_dram_parameter("fino", [M, D], i8, isOutput=True)
    souto = nc.declare_dram_parameter("souto", [128, KT], f32, isOutput=True)

    wqc = nc.inline_tensor(consts["wqp"], name="wqc")   # [D, D] bf16
    wkc = nc.inline_tensor(consts["wkp"], name="wkc")
    wvc = nc.inline_tensor(consts["wvt"], name="wvc")
    woc = nc.inline_tensor(consts["wo"], name="woc")
    bqc = nc.inline_tensor(consts["bqp"], name="bqc")   # [128, KT] f32
    bkc = nc.inline_tensor(consts["bkp"], name="bkc")
    boc = nc.inline_tensor(consts["bor"], name="boc")   # [128, D] f32
    eyec = nc.inline_tensor(np.eye(128, dtype=bfdt), name="eyec")

    with tile_mod.TileContext(nc) as tc:
        with (
            nc.allow_low_precision(reason="bf16 pipeline by design"),
            tc.tile_pool(name="wpool", bufs=1) as wpool,
            tc.tile_pool(name="x8pool", bufs=1) as x8pool,
            tc.tile_pool(name="xbfpool", bufs=1) as xbfpool,
            tc.tile_pool(name="xTpool", bufs=2) as xTpool,
            tc.tile_pool(name="persist", bufs=1) as ppool,
            tc.tile_pool(name="scratch", bufs=2) as spool,
            tc.tile_pool(name="zpool", bufs=1) as zpool,
            tc.tile_pool(name="qkpool", bufs=1) as qkpool,
            tc.tile_pool(name="pst", bufs=2, space="PSUM") as pstp,
            tc.tile_pool(name="psbig", bufs=2, space="PSUM") as psbig,
            tc.tile_pool(name="psbroad", bufs=1, space="PSUM") as psbroad,
            tc.tile_pool(name="pssmall", bufs=1, space="PSUM") as pssmall,
            tc.tile_pool(name="psd", bufs=1, space="PSUM") as psdp,
        ):
            # ---- persistent constants in SBUF ----
            eye = ppool.tile([128, 128], bf16, tag="eye")
            nc.sync.dma_start(eye[:], eyec[:, :])
            bq_sb = ppool.tile([128, KT], f32, tag="bq")
            bk_sb = ppool.tile([128, KT], f32, tag="bk")
            nc.sync.dma_start(bq_sb[:], bqc[:, :])
            nc.sync.dma_start(bk_sb[:], bkc[:, :])
            bo_sb = ppool.tile([128, D], bf16, tag="bo")
            nc.sync.dma_start(bo_sb[:], boc[:, :])
            sv_sb = ppool.tile([128, 16], f32, tag="sv")
            nc.sync.dma_start(sv_sb[:], svt[:, :])
            ones_col = ppool.tile([128, 1], bf16, tag="onec")
            nc.vector.memset(ones_col[:], 1.0)
            ones_row = ppool.tile([1, 128], bf16, tag="oner")
            nc.vector.memset(ones_row[:], 1.0)

            ksum = ppool.tile([128, KT, BL], f32, tag="ksum")
            ksum_bf = ppool.tile([128, KT, BL], bf16, tag="ksumbf")
            ut_sb = ppool.tile([128, KT, 64], bf16, tag="ut")
            srow = ppool.tile([128, 2, BL, H], f32, tag="srow")
            outT = ppool.tile([128, KT, M], bf16, tag="outT")     # 8MB

            def transpose_strip(xdram, b, tag="xT"):
                """natural int8 strip [256 tok, D] -> bf16 [128 ch, KT, 256 tok]."""
                x8 = x8pool.tile([128, 2, D], i8, tag="x8")
                nc.sync.dma_start(
                    x8[:],
                    xdram.rearrange("(b two p) c -> p b two c", two=2, p=128)[:, b])
                xbf = xbfpool.tile([128, 2, D], bf16, tag="xbf")
                nc.vector.tensor_copy(xbf[:], x8[:])
                xT = xTpool.tile([128, KT, S], bf16, tag=tag)
                for two in range(2):
                    for ct in range(KT):
                        pst = pstp.tile([128, 128], bf16, tag="t")
                        nc.tensor.transpose(
                            pst[:], xbf[:, two, ct * 128:(ct + 1) * 128], eye[:])
                        nc.scalar.activation(
                            xT[:, ct, two * 128:(two + 1) * 128], pst[:], Copy)
                return xT

            # ---------------- phase B: Kk -> Ksum ----------------
            wk = wpool.tile([128, KT, D], bf16, tag="W")
            for i in range(4):
                nc.sync.dma_start(
                    wk[:, 4 * i:4 * (i + 1), :],
                    wkc.rearrange("(t p) m -> p t m", p=128)[:, 4 * i:4 * (i + 1), :])
            for b in range(BL):
                xT = transpose_strip(xk, b, "xT")
                for t in range(KT):
                    ps = psbig.tile([128, S], f32, tag="big")
                    for k in range(KT):
                        nc.tensor.matmul(ps[:], wk[:, k, t * 128:(t + 1) * 128],
                                         xT[:, k, :], start=(k == 0), stop=(k == KT - 1))
                    scr = spool.tile([128, S], bf16, tag="scr")
                    nc.scalar.activation(scr[:], ps[:], Relu,
                                         bias=bk_sb[:, t:t + 1],
                                         accum_out=ksum[:, t, b:b + 1])
            nc.vector.tensor_scalar(ksum_bf[:], ksum[:], S * EPS, None, Alu.add)

            # ---------------- U^T ----------------
            wv = wpool.tile([128, KT, D], bf16, tag="W")
            for i in range(4):
                nc.sync.dma_start(
                    wv[:, 4 * i:4 * (i + 1), :],
                    wvc.rearrange("(t p) m -> p t m", p=128)[:, 4 * i:4 * (i + 1), :])
            for ct in range(KT):
                psu = pssmall.tile([128, 64], f32, tag="small")
                for h in range(H):
                    for j in range(2):
                        t = 2 * h + j
                        nc.tensor.matmul(psu[:, h * 8:(h + 1) * 8],
                                         wv[:, t, ct * 128:(ct + 1) * 128],
                                         ksum_bf[:, t, :],
                                         start=(j == 0), stop=(j == 1))
                nc.vector.tensor_copy(ut_sb[:, ct, :], psu[:])

            # ---------------- SrowT (with per-token v scale) ----------------
            for b in range(BL):
                xT = transpose_strip(xv, b, "xT")
                for vch in range(2):
                    pst8 = pssmall.tile([128, 64], f32, tag="small")
                    pss = pst8[:, 0:H]
                    for ct in range(KT):
                        nc.tensor.matmul(pss, xT[:, ct, vch * 128:(vch + 1) * 128],
                                         ut_sb[:, ct, b::BL],
                                         start=(ct == 0), stop=(ct == KT - 1))
                    nc.vector.tensor_scalar(srow[:, vch, b, :], pss,
                                            sv_sb[:, 2 * b + vch:2 * b + vch + 1],
                                            None, Alu.mult)

            # ---------------- phase A: Qk -> outT ----------------
            wq = wpool.tile([128, KT, D], bf16, tag="W")
            for i in range(4):
                nc.sync.dma_start(
                    wq[:, 4 * i:4 * (i + 1), :],
                    wqc.rearrange("(t p) m -> p t m", p=128)[:, 4 * i:4 * (i + 1), :])
            for n in range(BL):
                NW = S
                xT = transpose_strip(xq, n, "xT")
                qk = qkpool.tile([128, KT, NW], bf16, tag="qk")
                for t in range(KT):
                    ps = psbig.tile([128, NW], f32, tag="big")
                    for k in range(KT):
                        nc.tensor.matmul(ps[:], wq[:, k, t * 128:(t + 1) * 128],
                                         xT[:, k, :], start=(k == 0), stop=(k == KT - 1))
                    nc.scalar.activation(qk[:, t, :], ps[:], Relu,
                                         bias=bq_sb[:, t:t + 1])
                # Z per head -> reciprocal
                zbfs = []
                for h in range(H):
                    psz = pssmall.tile([1, NW], f32, tag="psz")
                    for j in range(2):
                        nc.tensor.matmul(psz[:], ones_col[:], qk[:, 2 * h + j, :],
                                         start=(j == 0), stop=(j == 1))
                    ztmp = spool.tile([1, NW], f32, tag="ztmp")
                    nc.vector.tensor_scalar(ztmp[:], psz[:], F * EPS + EPS, None,
                                            Alu.add)
                    zbf = zpool.tile([1, NW], bf16, tag=f"zbf{h}")
                    nc.vector.reciprocal(zbf[:], ztmp[:])
                    zbfs.append(zbf)
                for t in range(KT):
                    h, fh = t // 2, t % 2
                    psb = psbroad.tile([128, NW], f32, tag="broad")
                    nc.tensor.matmul(psb[:], ones_row[:], zbfs[h][:],
                                     start=True, stop=True)
                    tmp = spool.tile([128, NW], f32, tag="tmp")
                    nc.vector.tensor_scalar(
                        tmp[:], qk[:, t, :],
                        EPS, srow[:, fh, n, h:h + 1], Alu.add, Alu.mult)
                    nc.vector.tensor_tensor(outT[:, t, n * NW:(n + 1) * NW],
                                            tmp[:], psb[:], Alu.mult)

            # ---------------- phase D: fino[m, :] = outT^T @ Wo + bo ----------------
            wos = wpool.tile([128, KT, D], bf16, tag="W")
            for i in range(4):
                nc.sync.dma_start(
                    wos[:, 4 * i:4 * (i + 1), :],
                    woc.rearrange("(t p) m -> p t m", p=128)[:, 4 * i:4 * (i + 1), :])
            sout_sb = ppool.tile([128, KT], f32, tag="sout")
            for mt in range(KT):
                fob = zpool.tile([128, 4, 512], bf16, tag="fob")
                for oc in range(4):
                    psd = psdp.tile([128, 512], f32, tag="d")
                    for k in range(KT):
                        nc.tensor.matmul(psd[:],
                                         outT[:, k, mt * 128:(mt + 1) * 128],
                                         wos[:, k, oc * 512:(oc + 1) * 512],
                                         start=(k == 0), stop=(k == KT - 1))
                    nc.vector.tensor_tensor(fob[:, oc, :], psd[:],
                                            bo_sb[:, oc * 512:(oc + 1) * 512],
                                            Alu.add)
                rmax = spool.tile([128, 1], f32, tag="rmax")
                nc.vector.tensor_reduce(rmax[:], fob[:], mybir.AxisListType.XY,
                                        Alu.max, apply_absolute_value=True)
                nc.vector.tensor_scalar(rmax[:], rmax[:], 1e-30, None, Alu.max)
                rinv = spool.tile([128, 1], f32, tag="rinv")
                nc.vector.reciprocal(rinv[:], rmax[:])
                nc.vector.tensor_scalar(rinv[:], rinv[:], 127.0, None, Alu.mult)
                q8t = zpool.tile([128, 4, 512], i8, tag="q8t")
                nc.vector.tensor_scalar(q8t[:], fob[:], rinv[:], None, Alu.mult)
                nc.sync.dma_start(
                    fino.rearrange("(mt p) c -> p mt c", p=128)[:, mt, :],
                    q8t[:])
                nc.vector.tensor_scalar(sout_sb[:, mt:mt + 1], rmax[:],
                                        1.0 / 127.0, None, Alu.mult)
            nc.sync.dma_start(souto[:, :], sout_sb[:])
    return nc


# ---------------------------------------------------------------------------
# cached fast exec path (replaces bass2jax.run_bass_via_pjrt for our nc)
# ---------------------------------------------------------------------------
_ST = {}


def _install_fast_exec(nc):
    import jax
    import jax.numpy as jnp
    import numpy as _np
    from jax.sharding import Mesh, PartitionSpec, NamedSharding
    from jax.experimental.shard_map import shard_map
    import concourse.mybir as mybir
    from concourse import bass2jax

    bass2jax.install_neuronx_cc_hook()

    assert nc.dbg_addr is None or not nc.dbg_callbacks
    extra_dbg = {}
    if nc.dbg_addr is not None:
        extra_dbg[nc.dbg_addr.name] = np.zeros((1, 2), np.uint32)

    partition_name = nc.partition_id_tensor.name if nc.partition_id_tensor else None
    in_names, out_names, out_avals, out_shapes = [], [], [], []
    for alloc in nc.m.functions[0].allocations:
        if not isinstance(alloc, mybir.MemoryLocationSet):
            continue
        name = alloc.memorylocations[0].name
        if alloc.kind == "ExternalInput":
            if name != partition_name:
                in_names.append(name)
        elif alloc.kind == "ExternalOutput":
            out_names.append(name)
            shape = tuple(alloc.tensor_shape)
            dtype = mybir.dt.np(alloc.dtype)
            out_avals.append(jax.core.ShapedArray(shape, dtype))
            out_shapes.append((shape, dtype))
    n_params = len(in_names)
    n_outs = len(out_avals)
    in_names_full = in_names + out_names + ([partition_name] if partition_name else [])

    def _body(*args):
        operands = list(args)
        if partition_name is not None:
            operands.append(bass2jax.partition_id_tensor())
        outs = bass2jax._bass_exec_p.bind(
            *operands,
            out_avals=tuple(out_avals),
            in_names=tuple(in_names_full),
            out_names=tuple(out_names),
            lowering_input_output_aliases=(),
            sim_require_finite=True,
            sim_require_nnan=True,
            nc=nc,
        )
        return tuple(outs)

    devs = jax.devices()[:NCORES]
    mesh = Mesh(np.asarray(devs), ("core",))
    sharded = jax.jit(
        shard_map(_body, mesh=mesh,
                  in_specs=(PartitionSpec("core"),) * (n_params + n_outs),
                  out_specs=(PartitionSpec("core"),) * n_outs, check_rep=False),
        donate_argnums=tuple(range(n_params, n_params + n_outs)),
        keep_unused=True,
    )
    zsh = NamedSharding(mesh, PartitionSpec("core"))
    # initial carry for the donated output operands; after the first call the
    # previous call's output buffers are donated back (single-model ping-pong,
    # no allocation churn, no per-call zeros)
    carry = [
        jax.jit(lambda s=s, d=d: jnp.zeros((NCORES * s[0], *s[1:]), d),
                out_shardings=zsh)()
        for (s, d) in out_shapes
    ]

    _ST.update(dict(nc=nc, sharded=sharded, carry=carry, in_names=in_names,
                    out_names=out_names, out_shapes=out_shapes, zsh=zsh,
                    jax=jax, extra_dbg=extra_dbg, stash={}))

    if not getattr(bass2jax, "_fast_exec_installed", False):
        _orig = bass2jax.run_bass_via_pjrt

        def _fast_run(nc_arg, in_maps, n_cores):
            if nc_arg is not _ST.get("nc") or n_cores != NCORES:
                return _orig(nc_arg, in_maps, n_cores)
            st = _ST
            ins = []
            for nm in st["in_names"]:
                if nm in st["extra_dbg"]:
                    ins.append(_np.concatenate([st["extra_dbg"][nm]] * n_cores, 0))
                    continue
                parts = [m[nm] for m in in_maps]
                stash = st["stash"].get(nm)
                if stash is not None and all(
                        parts[c] is stash[1][c] for c in range(n_cores)):
                    ins.append(stash[0])
                else:
                    ins.append(_np.concatenate(parts, axis=0))
            # explicit upload + delete: device input buffers freed eagerly
            # (lazy frees otherwise accumulate and degrade later dispatches)
            jx = st["jax"]
            ins_dev = [jx.device_put(a, st["zsh"]) for a in ins]
            outs = st["sharded"](*ins_dev, *st["carry"])
            outs_np = [_np.asarray(o) for o in outs]
            st["carry"] = list(outs)
            for a in ins_dev:
                a.delete()
            results = []
            for c in range(n_cores):
                r = {}
                for i, nm in enumerate(st["out_names"]):
                    s0 = st["out_shapes"][i][0][0]
                    r[nm] = outs_np[i][c * s0:(c + 1) * s0]
                results.append(r)
            return results

        bass2jax.run_bass_via_pjrt = _fast_run
        bass2jax._fast_exec_installed = True


def _fold_consts(Wq, bq, Wk, bk, Wv, bv, Wo, bo, random_proj, s_k):
    Wq4 = Wq.reshape(D, H, DK)
    Wqp = np.einsum('dhk,kf->dhf', Wq4, random_proj).reshape(D, D)
    bqp = (bq.reshape(H, DK) @ random_proj).reshape(D).astype(np.float32)
    Wk4 = Wk.reshape(D, H, DK)
    Wkp = (np.einsum('dhk,kf->dhf', Wk4, random_proj) * s_k).reshape(D, D)
    bkp = (bk.reshape(H, DK) @ random_proj).reshape(D).astype(np.float32)
    return {
        "wqp": np.ascontiguousarray(Wqp).astype(bfdt),
        "wkp": np.ascontiguousarray(Wkp).astype(bfdt),
        "wvt": np.ascontiguousarray(Wv.T).astype(bfdt),
        "wo": np.ascontiguousarray(Wo).astype(bfdt),
        "bqp": np.ascontiguousarray(bqp.reshape(KT, 128).T).astype(np.float32),
        "bkp": np.ascontiguousarray(bkp.reshape(KT, 128).T).astype(np.float32),
        "bor": np.ascontiguousarray(
            np.broadcast_to(bo, (128, D))).astype(bfdt),
    }


def _quant_tok(x):
    """int8 per-token (row) quantization. x [N, D] f32 -> (int8, scales f32)."""
    s = np.abs(x).max(axis=1) / 127.0
    np.maximum(s, 1e-30, out=s)
    q = np.rint(x * (1.0 / s)[:, None])
    np.clip(q, -127, 127, out=q)
    return q.astype(np.int8), s.astype(np.float32)


def kernel(query, key, value, Wq, bq, Wk, bk, Wv, bv, Wo, bo, random_proj):
    from concourse.bass_utils import run_bass_kernel_spmd

    query = np.asarray(query, dtype=np.float32)
    key = np.asarray(key, dtype=np.float32)
    value = np.asarray(value, dtype=np.float32)

    assert not np.any(np.asarray(bq)), "kernel assumes bq == 0 (q-scale fold)"
    assert not np.any(np.asarray(bv)), "kernel assumes bv == 0 (Srow bv term)"

    s_k = float(np.abs(key).max()) / 127.0

    h = hashlib.sha1()
    for a in (Wq, bq, Wk, bk, Wv, bv, Wo, bo, random_proj):
        h.update(np.ascontiguousarray(a).tobytes())
    h.update(np.float64(s_k).tobytes())
    fp = h.hexdigest()

    if _ST.get("fp") != fp:
        consts = _fold_consts(np.asarray(Wq), np.asarray(bq), np.asarray(Wk),
                              np.asarray(bk), np.asarray(Wv), np.asarray(bv),
                              np.asarray(Wo), np.asarray(bo),
                              np.asarray(random_proj), s_k)
        nc = _build(consts)
        _install_fast_exec(nc)
        _ST["fp"] = fp

    nc = _ST["nc"]

    # quantize (host, one-time per call)
    q8, _ = _quant_tok(query.reshape(B * S, D))
    k8 = np.clip(np.rint(key.reshape(B * S, D) * (1.0 / s_k)),
                 -127, 127).astype(np.int8)
    v8, sv = _quant_tok(value.reshape(B * S, D))
    # sv arranged per core as [128, BL*2]: col = b*2 + vch, row = token%128
    sv_arr = sv.reshape(NCORES, BL, 2, 128).transpose(0, 3, 1, 2).reshape(
        NCORES * 128, BL * 2).astype(np.float32)
    sv_arr = np.ascontiguousarray(sv_arr)

    globals_map = {"xq": q8, "xk": k8, "xv": v8, "svt": sv_arr}
    in_maps = []
    views = {nm: [] for nm in globals_map}
    for c in range(NCORES):
        im = {}
        for nm, g in globals_map.items():
            s0 = g.shape[0] // NCORES
            v = g[c * s0:(c + 1) * s0]
            im[nm] = v
            views[nm].append(v)
        in_maps.append(im)
    for nm, g in globals_map.items():
        _ST["stash"][nm] = (g, views[nm])

    res = run_bass_kernel_spmd(nc, in_maps, list(range(NCORES)))
    kernel._last_in_maps = in_maps

    fino_g = np.concatenate([res.results[c]["fino"] for c in range(NCORES)], 0)
    souto_g = np.concatenate([res.results[c]["souto"] for c in range(NCORES)], 0)
    s_tok = souto_g.reshape(NCORES, 128, KT).transpose(0, 2, 1).reshape(B * S, 1)
    out = (fino_g.astype(np.float32) * s_tok).reshape(B, S, D)
    return out
